# revision 1
# baseline (speedup 1.0000x reference)
"""Trainium2 Bass kernel for nn_EnhancedChunkLayer (ragged_sequence).

Strategy: data-parallel over batch B=8 across 8 NeuronCores (one batch
element per core, weights replicated). Inside each core:
  - banded block-diagonal attention: chunk segments are contiguous, so each
    128-row tile of scores only needs a W-wide column window around the
    diagonal. Host computes an additive mask (0 / -30000) from `boundaries`.
  - softmax without max-subtraction (scores are O(1) here), denominator via
    ACT accum_out, per-row normalize on DVE, PE transposes for attn @ V.
  - segment mean-pool via one-hot matmul (O'^T @ attn_out), counts folded
    into the out-proj bias via a K=1 rank-1 matmul.
  - size-embedding lookup via one-hot matmul; chunk MLP + exact-erf GELU +
    LayerNorm on chip.
  - all large GEMMs run in float32r (1 cycle/row vs 4 for fp32 on the PE;
    measured 1.3e-4 rel on a 1536-deep contraction); attention prob @ V and
    transposes stay fp32.

The host only does index bookkeeping (cumsum of boundary indicators,
one-hot/mask construction, weight transposes for layout); every FLOP on the
[S,D]-sized tensors runs on the NeuronCores.
"""

import math
from contextlib import ExitStack

import numpy as np

import concourse.bacc as bacc
import concourse.bass as bass
import concourse.mybir as mybir
from concourse import tile
from concourse.bass_utils import run_bass_kernel_spmd

F32 = mybir.dt.float32
F32R = mybir.dt.float32r
AF = mybir.ActivationFunctionType
ALU = mybir.AluOpType
AX = mybir.AxisListType

B, S, D = 8, 1024, 1536
H, DH = 12, 128
MAXC, MAXSEQ = 256, 1024
THRESH = 0.85
P = 128
KD = D // P          # 12 contraction tiles over D
NT = S // P          # 8 row tiles over S
N2 = (2 * D) // P    # 24 tiles over hidden 2D
CT = MAXC // P       # 2 chunk tiles
DD3 = D // 512       # 3 free-dim 512 tiles over D
LT8 = MAXSEQ // P    # 8 tiles over the size-embedding table
INV_SD = 1.0 / math.sqrt(DH)
NEG = -30000.0


# ---------------------------------------------------------------- host prep

def _host_segments(boundaries_b):
    is_b = boundaries_b > THRESH
    seg = np.cumsum(is_b.astype(np.int64)) - 1
    valid = seg >= 0
    seg_c = np.where(valid & (seg < MAXC), seg, MAXC)
    lengths = np.bincount(seg_c, minlength=MAXC + 1)[:MAXC]
    return seg, valid, seg_c, lengths


def _window_tiles(seg_list):
    """Smallest odd tile-count window covering every chunk from any row tile."""
    wt = 3
    while True:
        if wt > NT:
            return NT
        pad = (wt - 1) // 2
        ok = True
        for seg in seg_list:
            for t in range(NT):
                ci = min(max(t - pad, 0), NT - wt)
                lo, hi = ci * P, ci * P + wt * P
                rows = np.arange(t * P, (t + 1) * P)
                segs = seg[rows]
                vmask = segs >= 0
                if not vmask.any():
                    continue
                cols = np.isin(seg, segs[vmask]) & (seg >= 0)
                idx = np.nonzero(cols)[0]
                if len(idx) and (idx[0] < lo or idx[-1] >= hi):
                    ok = False
                    break
            if not ok:
                break
        if ok:
            return wt
        wt += 2


def _host_per_batch(seg, valid, seg_c, lengths, wt):
    Wc = wt * P
    pad = (wt - 1) // 2
    maskbias = np.full((S, Wc), NEG, dtype=np.float32)
    for t in range(NT):
        ci = min(max(t - pad, 0), NT - wt)
        rows = slice(t * P, (t + 1) * P)
        seg_r = seg[rows]
        seg_w = seg[ci * P: ci * P + Wc]
        m = (seg_r[:, None] == seg_w[None, :]) & (seg_r >= 0)[:, None]
        maskbias[rows][m] = 0.0
    oprime = np.zeros((S, MAXC), dtype=np.float32)
    ok = seg_c < MAXC
    oprime[np.arange(S)[ok], seg_c[ok]] = 1.0
    soT = np.zeros((MAXSEQ, MAXC), dtype=np.float32)
    ne = lengths > 0
    soT[np.minimum(lengths[ne], MAXSEQ - 1), np.nonzero(ne)[0]] = 1.0
    lens_row = lengths.astype(np.float32)[None, :]
    recip = (1.0 / np.maximum(lengths, 1)).astype(np.float32)
    return maskbias, oprime, soT, lens_row, recip


# ------------------------------------------------------------- device build

def build_nc(wt, sim_safe=False, repeat=1):
    """Build the per-core Bass program for window width wt*128 columns.

    sim_safe: replace Gelu (unimplemented in CoreSim) with Identity.
    repeat: emit the whole pipeline N times (for slope-based HW timing)."""
    Wc = wt * P
    pad = (wt - 1) // 2
    ct_idx = [min(max(t - pad, 0), NT - wt) for t in range(NT)]
    MR = F32R  # matmul operand dtype for the big GEMMs

    nc = bacc.Bacc("TRN2", target_bir_lowering=False, debug=False)
    dp = nc.declare_dram_parameter
    dram = {
        "xT": dp("xT", [D, S], MR, isOutput=False),
        "wqT": dp("wqT", [D, D], MR, isOutput=False),
        "wkT": dp("wkT", [D, D], MR, isOutput=False),
        "wvT": dp("wvT", [D, D], MR, isOutput=False),
        "woT": dp("woT", [D, D], MR, isOutput=False),
        "w1T": dp("w1T", [D, 2 * D], MR, isOutput=False),
        "w2T": dp("w2T", [2 * D, D], MR, isOutput=False),
        "bq": dp("bq", [D], F32, isOutput=False),
        "bk": dp("bk", [D], F32, isOutput=False),
        "bv": dp("bv", [D], MR, isOutput=False),
        "ob": dp("ob", [D], MR, isOutput=False),
        "b1": dp("b1", [2 * D], F32, isOutput=False),
        "b2": dp("b2", [D], MR, isOutput=False),
        "lng": dp("lng", [D], MR, isOutput=False),
        "lnb": dp("lnb", [D], MR, isOutput=False),
        "pe": dp("pe", [MAXC, D], F32, isOutput=False),
        "se": dp("se", [MAXSEQ, D], MR, isOutput=False),
        "maskbias": dp("maskbias", [S, Wc], F32, isOutput=False),
        "oprime": dp("oprime", [S, MAXC], MR, isOutput=False),
        "soT": dp("soT", [MAXSEQ, MAXC], MR, isOutput=False),
        "lens_row": dp("lens_row", [1, MAXC], MR, isOutput=False),
        "recip": dp("recip", [MAXC], F32, isOutput=False),
        "ident": dp("ident", [P, P], F32, isOutput=False),
        "ones": dp("ones", [1, P], MR, isOutput=False),
        "out": dp("out", [MAXC, D], F32, isOutput=True),
    }

    with ExitStack() as octx:
        tc = octx.enter_context(tile.TileContext(nc))
        for _rep in range(repeat):
            _emit(nc, tc, wt, Wc, ct_idx, MR, sim_safe, dram)

    nc.finalize()
    return nc


def _emit(nc, tc, wt, Wc, ct_idx, MR, sim_safe, dram):
    d = dram
    with ExitStack() as ctx:
        persist = ctx.enter_context(tc.tile_pool(name="persist", bufs=1))
        psum512 = ctx.enter_context(
            tc.tile_pool(name="psum512", bufs=2, space="PSUM"))

        # ---- persistent small tensors
        ident = persist.tile([P, P], F32, tag="ident")
        nc.sync.dma_start(ident[:], d["ident"].ap()[:])
        ones_row = persist.tile([1, P], MR, tag="ones")
        nc.sync.dma_start(ones_row[:], d["ones"].ap()[:])
        bq_sb = persist.tile([P, KD], F32, tag="bq")
        nc.sync.dma_start(bq_sb[:], d["bq"].ap().rearrange("(k p) -> p k", p=P))
        nc.vector.tensor_scalar_mul(bq_sb[:], bq_sb[:], INV_SD)
        bk_sb = persist.tile([P, KD], F32, tag="bk")
        nc.sync.dma_start(bk_sb[:], d["bk"].ap().rearrange("(k p) -> p k", p=P))
        b1_sb = persist.tile([P, N2], F32, tag="b1")
        nc.sync.dma_start(b1_sb[:], d["b1"].ap().rearrange("(k p) -> p k", p=P))
        lens_row = persist.tile([1, MAXC], MR, tag="lens")
        nc.sync.dma_start(lens_row[:], d["lens_row"].ap()[:])
        recip_sb = persist.tile([P, CT], F32, tag="recip")
        nc.sync.dma_start(recip_sb[:],
                          d["recip"].ap().rearrange("(c p) -> p c", p=P))
        eps_sb = persist.tile([P, 1], F32, tag="eps")
        nc.vector.memset(eps_sb[:], 1e-5)

        ctxT = persist.tile([P, H * S], MR, tag="ctxT")

        # ================= stages 0-2 (need xT and V resident) =============
        with tc.tile_pool(name="xv", bufs=1) as xv, \
             tc.tile_pool(name="small", bufs=4) as smallp:

            xT = xv.tile([P, KD * S], MR, tag="xT")
            for kd in range(KD):
                nc.sync.dma_start(xT[:, kd * S:(kd + 1) * S],
                                  d["xT"].ap()[kd * P:(kd + 1) * P, :])
            v_sb = xv.tile([P, NT * D], F32, tag="v")
            maskb = xv.tile([P, NT * Wc], F32, tag="maskb")
            for t in range(NT):
                nc.sync.dma_start(maskb[:, t * Wc:(t + 1) * Wc],
                                  d["maskbias"].ap()[t * P:(t + 1) * P, :])

            # ---- stage 1: V = x @ Wv.T + bv, token-major [tok, feat]
            with tc.tile_pool(name="wvs", bufs=2) as wvs, \
                 tc.tile_pool(name="psum_v", bufs=4, space="PSUM") as psum_v:
                bv_row = wvs.tile([1, D], MR, tag="bv", bufs=1)
                nc.sync.dma_start(bv_row[:],
                                  d["bv"].ap().rearrange("(o d) -> o d", o=1))
                for nt3 in range(DD3):
                    wvh = []
                    for kh in range(2):
                        wtl = wvs.tile([P, 6 * 512], MR, tag="wv",
                                       name=f"wv{nt3}_{kh}")
                        nc.sync.dma_start(
                            wtl[:].rearrange("p (k c) -> p k c", c=512),
                            d["wvT"].ap()[kh * 6 * P:(kh + 1) * 6 * P,
                                          nt3 * 512:(nt3 + 1) * 512]
                            .rearrange("(k p) c -> p k c", p=P))
                        wvh.append(wtl)
                    for mt in range(NT):
                        pv = psum_v.tile([P, 512], F32, tag="pv")
                        for kd in range(KD):
                            nc.tensor.matmul(
                                pv[:],
                                xT[:, kd * S + mt * P: kd * S + (mt + 1) * P],
                                wvh[kd // 6][:, (kd % 6) * 512:(kd % 6 + 1) * 512],
                                start=(kd == 0), stop=False)
                        nc.tensor.matmul(pv[:], ones_row[:],
                                         bv_row[:, nt3 * 512:(nt3 + 1) * 512],
                                         start=False, stop=True)
                        nc.vector.tensor_copy(
                            v_sb[:, mt * D + nt3 * 512: mt * D + (nt3 + 1) * 512],
                            pv[:])

            # ---- stage 2: per-head attention
            with tc.tile_pool(name="wqks", bufs=2) as wqks, \
                 tc.tile_pool(name="qk", bufs=1) as qkp, \
                 tc.tile_pool(name="attnwork", bufs=2) as aw, \
                 tc.tile_pool(name="attnT", bufs=wt + 1) as atp, \
                 tc.tile_pool(name="psum_sc", bufs=2 if wt <= 3 else 1,
                              space="PSUM") as psum_sc, \
                 tc.tile_pool(name="psum_tp", bufs=2, space="PSUM") as psum_tp, \
                 tc.tile_pool(name="psum_cx", bufs=2, space="PSUM") as psum_cx:
                for h in range(H):
                    qt = qkp.tile([P, S], MR, tag="qt")
                    kt = qkp.tile([P, S], MR, tag="kt")
                    for name, wd, dst in (("q", d["wqT"], qt),
                                          ("k", d["wkT"], kt)):
                        wtl = wqks.tile([P, KD * P], MR, tag="wqk",
                                        name=f"w{name}{h}")
                        nc.sync.dma_start(
                            wtl[:].rearrange("p (k c) -> p k c", c=P),
                            wd.ap()[:, h * P:(h + 1) * P]
                            .rearrange("(k p) c -> p k c", p=P))
                        for mt2 in range(2):
                            pq = psum512.tile([P, 512], F32, tag="mm512")
                            for kd in range(KD):
                                nc.tensor.matmul(
                                    pq[:], wtl[:, kd * P:(kd + 1) * P],
                                    xT[:, kd * S + mt2 * 512:
                                       kd * S + (mt2 + 1) * 512],
                                    start=(kd == 0), stop=(kd == KD - 1))
                            if name == "q":
                                nc.vector.tensor_scalar(
                                    dst[:, mt2 * 512:(mt2 + 1) * 512], pq[:],
                                    INV_SD, bq_sb[:, h:h + 1], ALU.mult, ALU.add)
                            else:
                                nc.vector.tensor_scalar_add(
                                    dst[:, mt2 * 512:(mt2 + 1) * 512], pq[:],
                                    bk_sb[:, h:h + 1])
                    for t in range(NT):
                        ci = ct_idx[t]
                        ps = psum_sc.tile([P, Wc], F32, tag="ps")
                        nc.tensor.matmul(ps[:], qt[:, t * P:(t + 1) * P],
                                         kt[:, ci * P: ci * P + Wc],
                                         start=True, stop=True)
                        sm = aw.tile([P, Wc], F32, tag="sm")
                        nc.vector.tensor_tensor(
                            sm[:], ps[:], maskb[:, t * Wc:(t + 1) * Wc], ALU.add)
                        en = aw.tile([P, Wc], F32, tag="en")
                        den = smallp.tile([P, 1], F32, tag="den")
                        nc.scalar.activation(en[:], sm[:], AF.Exp,
                                             accum_out=den[:])
                        den2 = smallp.tile([P, 1], F32, tag="den2")
                        nc.vector.tensor_scalar_max(den2[:], den[:], 1e-30)
                        rden = smallp.tile([P, 1], F32, tag="rden")
                        nc.vector.reciprocal(rden[:], den2[:])
                        at = aw.tile([P, Wc], F32, tag="at")
                        nc.vector.tensor_scalar_mul(at[:], en[:], rden[:])
                        atts = []
                        for w in range(wt):
                            ptp = psum_tp.tile([P, P], F32, tag="ptp")
                            nc.tensor.transpose(ptp[:],
                                                at[:, w * P:(w + 1) * P],
                                                ident[:])
                            atT = atp.tile([P, P], F32, tag="atT")
                            nc.vector.tensor_copy(atT[:], ptp[:])
                            atts.append(atT)
                        pc = psum_cx.tile([P, P], F32, tag="pc")
                        for w in range(wt):
                            nc.tensor.matmul(
                                pc[:],
                                v_sb[:, (ci + w) * D + h * P:
                                     (ci + w) * D + (h + 1) * P],
                                atts[w][:], start=(w == 0), stop=(w == wt - 1))
                        nc.vector.tensor_copy(
                            ctxT[:, h * S + t * P: h * S + (t + 1) * P], pc[:])

        # ================= stage 3: out-proj (+bias) =======================
        with tc.tile_pool(name="post", bufs=1) as post:
            ob_row = post.tile([1, D], MR, tag="ob")
            nc.sync.dma_start(ob_row[:],
                              d["ob"].ap().rearrange("(o d) -> o d", o=1))
            b2_row = post.tile([1, D], MR, tag="b2")
            nc.sync.dma_start(b2_row[:],
                              d["b2"].ap().rearrange("(o d) -> o d", o=1))
            pe_sb = post.tile([P, CT * D], F32, tag="pe")
            for c in range(CT):
                nc.sync.dma_start(pe_sb[:, c * D:(c + 1) * D],
                                  d["pe"].ap()[c * P:(c + 1) * P, :])
            chunks = post.tile([P, CT * D], F32, tag="chunks")
            cT = post.tile([P, KD * MAXC], MR, tag="cT")
            mlp_ctx = ExitStack()

            with tc.tile_pool(name="aop", bufs=1) as aop:
                ao = aop.tile([P, NT * D], MR, tag="ao")
                with tc.tile_pool(name="wstream2", bufs=2) as ws2, \
                     tc.tile_pool(name="psum_o", bufs=4, space="PSUM") as psum_o:
                    for dd3 in range(DD3):
                        woh = []
                        for kh in range(2):
                            wtl = ws2.tile([P, 6 * 512], MR, tag="wo",
                                           name=f"wo{dd3}_{kh}")
                            nc.sync.dma_start(
                                wtl[:].rearrange("p (k c) -> p k c", c=512),
                                d["woT"].ap()[kh * 6 * P:(kh + 1) * 6 * P,
                                              dd3 * 512:(dd3 + 1) * 512]
                                .rearrange("(k p) c -> p k c", p=P))
                            woh.append(wtl)
                        for mt in range(NT):
                            po = psum_o.tile([P, 512], F32, tag="po")
                            for kd in range(KD):
                                nc.tensor.matmul(
                                    po[:],
                                    ctxT[:, kd * S + mt * P:
                                         kd * S + (mt + 1) * P],
                                    woh[kd // 6][:, (kd % 6) * 512:
                                                 (kd % 6 + 1) * 512],
                                    start=(kd == 0), stop=False)
                            nc.tensor.matmul(
                                po[:], ones_row[:],
                                ob_row[:, dd3 * 512:(dd3 + 1) * 512],
                                start=False, stop=True)
                            nc.vector.tensor_copy(
                                ao[:, mt * D + dd3 * 512:
                                   mt * D + (dd3 + 1) * 512], po[:])

                # ---- stage 4+5: pooled mean + size embedding + pos enc
                with tc.tile_pool(name="poolmm", bufs=2) as pm, \
                     tc.tile_pool(name="psum_pool", bufs=3,
                                  space="PSUM") as pspool:
                    for c in range(CT):
                        opt = pm.tile([P, NT * P], MR, tag="opt", name=f"opt{c}")
                        nc.sync.dma_start(
                            opt[:].rearrange("p (m c) -> p m c", c=P),
                            d["oprime"].ap()[:, c * P:(c + 1) * P]
                            .rearrange("(m p) c -> p m c", p=P))
                        mean_ps = [pspool.tile([P, 512], F32, tag="sums",
                                               name=f"sums{c}_{i}")
                                   for i in range(DD3)]
                        for mt in range(NT):
                            for dd3 in range(DD3):
                                nc.tensor.matmul(
                                    mean_ps[dd3][:],
                                    opt[:, mt * P:(mt + 1) * P],
                                    ao[:, mt * D + dd3 * 512:
                                       mt * D + (dd3 + 1) * 512],
                                    start=(mt == 0), stop=False)
                        for dd3 in range(DD3):
                            nc.tensor.matmul(
                                mean_ps[dd3][:], lens_row[:, c * P:(c + 1) * P],
                                ob_row[:, dd3 * 512:(dd3 + 1) * 512],
                                start=False, stop=True)
                        sot = pm.tile([P, LT8 * P], MR, tag="sot", name=f"sot{c}")
                        nc.sync.dma_start(
                            sot[:].rearrange("p (l c) -> p l c", c=P),
                            d["soT"].ap()[:, c * P:(c + 1) * P]
                            .rearrange("(l p) c -> p l c", p=P))
                        se_ps = [pspool.tile([P, 512], F32, tag="sev",
                                             name=f"sev{c}_{i}")
                                 for i in range(DD3)]
                        for lt in range(LT8):
                            ser = pm.tile([P, D], MR, tag="ser",
                                          name=f"ser{c}_{lt}")
                            nc.sync.dma_start(
                                ser[:], d["se"].ap()[lt * P:(lt + 1) * P, :])
                            for dd3 in range(DD3):
                                nc.tensor.matmul(
                                    se_ps[dd3][:], sot[:, lt * P:(lt + 1) * P],
                                    ser[:, dd3 * 512:(dd3 + 1) * 512],
                                    start=(lt == 0), stop=(lt == LT8 - 1))
                        for dd3 in range(DD3):
                            sl = slice(c * D + dd3 * 512,
                                       c * D + (dd3 + 1) * 512)
                            nc.vector.tensor_scalar_mul(
                                chunks[:, sl], mean_ps[dd3][:],
                                recip_sb[:, c:c + 1])
                            nc.vector.tensor_tensor(
                                chunks[:, sl], chunks[:, sl], se_ps[dd3][:],
                                ALU.add)
                            nc.vector.tensor_tensor(
                                chunks[:, sl], chunks[:, sl], pe_sb[:, sl],
                                ALU.add)

            mlp = mlp_ctx.enter_context(tc.tile_pool(name="mlp", bufs=1))
            # ---- stage 6: transpose chunks -> cT [D, MAXC]
            with tc.tile_pool(name="psum_tp2", bufs=4, space="PSUM") as ptp2:
                for c in range(CT):
                    for kd in range(KD):
                        pt = ptp2.tile([P, P], F32, tag="pt2")
                        nc.tensor.transpose(
                            pt[:],
                            chunks[:, c * D + kd * P: c * D + (kd + 1) * P],
                            ident[:])
                        nc.vector.tensor_copy(
                            cT[:, kd * MAXC + c * P: kd * MAXC + (c + 1) * P],
                            pt[:])

            # ---- stage 7: h1T = gelu(W1 @ chunks^T + b1), [2D, MAXC]
            h1T = mlp.tile([P, N2 * MAXC], MR, tag="h1T")
            with tc.tile_pool(name="w1s", bufs=3) as w1s, \
                 tc.tile_pool(name="psum_h1", bufs=2, space="PSUM") as ph1p:
                for n in range(N2):
                    w1t = w1s.tile([P, KD * P], MR, tag="w1t", name=f"w1t{n}")
                    nc.sync.dma_start(
                        w1t[:].rearrange("p (k c) -> p k c", c=P),
                        d["w1T"].ap()[:, n * P:(n + 1) * P]
                        .rearrange("(k p) c -> p k c", p=P))
                    ph = ph1p.tile([P, MAXC], F32, tag="ph1")
                    for kd in range(KD):
                        nc.tensor.matmul(
                            ph[:], w1t[:, kd * P:(kd + 1) * P],
                            cT[:, kd * MAXC:(kd + 1) * MAXC],
                            start=(kd == 0), stop=(kd == KD - 1))
                    nc.scalar.activation(
                        h1T[:, n * MAXC:(n + 1) * MAXC], ph[:],
                        AF.Identity if sim_safe else AF.Gelu,
                        bias=b1_sb[:, n:n + 1])

            # ---- stage 8: h2 = h1 @ W2.T + b2, [MAXC, D] + layernorm
            with tc.tile_pool(name="w2s", bufs=3) as w2s, \
                 tc.tile_pool(name="ln", bufs=1) as lnp, \
                 tc.tile_pool(name="lnsmall", bufs=6) as lns, \
                 tc.tile_pool(name="psum_h2", bufs=6, space="PSUM") as ph2p:
                lng_row = lnp.tile([1, D], MR, tag="lngr")
                nc.sync.dma_start(lng_row[:],
                                  d["lng"].ap().rearrange("(o d) -> o d", o=1))
                lnb_row = lnp.tile([1, D], MR, tag="lnbr")
                nc.sync.dma_start(lnb_row[:],
                                  d["lnb"].ap().rearrange("(o d) -> o d", o=1))
                lng_b = mlp.tile([P, D], F32, tag="lngb")
                lnb_b = mlp.tile([P, D], F32, tag="lnbb")
                for dd3 in range(DD3):
                    for row, dst in ((lng_row, lng_b), (lnb_row, lnb_b)):
                        pb = psum512.tile([P, 512], F32, tag="mm512",
                                          name=f"pbc{dd3}")
                        nc.tensor.matmul(pb[:], ones_row[:],
                                         row[:, dd3 * 512:(dd3 + 1) * 512],
                                         start=True, stop=True)
                        nc.vector.tensor_copy(
                            dst[:, dd3 * 512:(dd3 + 1) * 512], pb[:])

                h2_ps = [[ph2p.tile([P, 512], F32, tag="ph2",
                                    name=f"ph2_{i}_{j}")
                          for j in range(DD3)] for i in range(CT)]
                for n in range(N2):
                    w2r = w2s.tile([P, D], MR, tag="w2r", name=f"w2r{n}")
                    nc.sync.dma_start(w2r[:],
                                      d["w2T"].ap()[n * P:(n + 1) * P, :])
                    for c in range(CT):
                        for dd3 in range(DD3):
                            nc.tensor.matmul(
                                h2_ps[c][dd3][:],
                                h1T[:, n * MAXC + c * P: n * MAXC + (c + 1) * P],
                                w2r[:, dd3 * 512:(dd3 + 1) * 512],
                                start=(n == 0), stop=False)
                for c in range(CT):
                    h2 = lnp.tile([P, D], F32, tag="h2")
                    for dd3 in range(DD3):
                        nc.tensor.matmul(
                            h2_ps[c][dd3][:], ones_row[:],
                            b2_row[:, dd3 * 512:(dd3 + 1) * 512],
                            start=False, stop=True)
                        nc.vector.tensor_copy(
                            h2[:, dd3 * 512:(dd3 + 1) * 512], h2_ps[c][dd3][:])
                    negsum = lns.tile([P, 1], F32, tag="negsum")
                    nc.vector.reduce_sum(negsum[:], h2[:], axis=AX.X,
                                         negate=True)
                    negmu = lns.tile([P, 1], F32, tag="negmu")
                    nc.vector.tensor_scalar_mul(negmu[:], negsum[:], 1.0 / D)
                    xm = lnp.tile([P, D], F32, tag="xm")
                    nc.vector.tensor_scalar_add(xm[:], h2[:], negmu[:])
                    ssq = lns.tile([P, 1], F32, tag="ssq")
                    sq = lnp.tile([P, D], F32, tag="sq")
                    nc.scalar.activation(sq[:], xm[:], AF.Square,
                                         accum_out=ssq[:])
                    std = lns.tile([P, 1], F32, tag="std")
                    nc.scalar.activation(std[:], ssq[:], AF.Sqrt,
                                         bias=eps_sb[:], scale=1.0 / D)
                    rstd = lns.tile([P, 1], F32, tag="rstd")
                    nc.vector.reciprocal(rstd[:], std[:])
                    y = lnp.tile([P, D], F32, tag="y")
                    nc.vector.scalar_tensor_tensor(
                        y[:], xm[:], rstd[:], lng_b[:], ALU.mult, ALU.mult)
                    yout = lnp.tile([P, D], F32, tag="yout")
                    nc.vector.tensor_tensor(yout[:], y[:], lnb_b[:], ALU.add)
                    nc.sync.dma_start(d["out"].ap()[c * P:(c + 1) * P, :],
                                      yout[:])
            mlp_ctx.close()


# ------------------------------------------------------------------ driver

def prepare_inputs(x, boundaries, in_proj_w, in_proj_b, out_w, out_b,
                   w1, b1, w2, b2, ln_g, ln_b, pos_enc, size_emb):
    """Host prep: returns (wt, in_maps) for the 8 cores."""
    x = np.ascontiguousarray(np.asarray(x, dtype=np.float32))
    boundaries = np.asarray(boundaries, dtype=np.float32)
    segs = [_host_segments(boundaries[b]) for b in range(B)]
    wt = _window_tiles([s[0] for s in segs])

    shared = {
        "wqT": np.ascontiguousarray(np.asarray(in_proj_w[0:D]).T.astype(np.float32)),
        "wkT": np.ascontiguousarray(np.asarray(in_proj_w[D:2 * D]).T.astype(np.float32)),
        "wvT": np.ascontiguousarray(np.asarray(in_proj_w[2 * D:3 * D]).T.astype(np.float32)),
        "woT": np.ascontiguousarray(np.asarray(out_w).T.astype(np.float32)),
        "w1T": np.ascontiguousarray(np.asarray(w1).T.astype(np.float32)),
        "w2T": np.ascontiguousarray(np.asarray(w2).T.astype(np.float32)),
        "bq": np.asarray(in_proj_b[0:D], dtype=np.float32),
        "bk": np.asarray(in_proj_b[D:2 * D], dtype=np.float32),
        "bv": np.asarray(in_proj_b[2 * D:3 * D], dtype=np.float32),
        "ob": np.asarray(out_b, dtype=np.float32),
        "b1": np.asarray(b1, dtype=np.float32),
        "b2": np.asarray(b2, dtype=np.float32),
        "lng": np.asarray(ln_g, dtype=np.float32),
        "lnb": np.asarray(ln_b, dtype=np.float32),
        "pe": np.ascontiguousarray(
            np.asarray(pos_enc, dtype=np.float32).reshape(MAXC, D)),
        "se": np.ascontiguousarray(np.asarray(size_emb, dtype=np.float32)),
        "ident": np.eye(P, dtype=np.float32),
        "ones": np.ones((1, P), dtype=np.float32),
    }
    in_maps = []
    for b in range(B):
        seg, valid, seg_c, lengths = segs[b]
        maskbias, oprime, soT, lens_row, recip = _host_per_batch(
            seg, valid, seg_c, lengths, wt)
        m = dict(shared)
        m["xT"] = np.ascontiguousarray(x[b].T)
        m["maskbias"] = maskbias
        m["oprime"] = oprime
        m["soT"] = soT
        m["lens_row"] = lens_row
        m["recip"] = recip
        in_maps.append(m)
    return wt, in_maps


_NC_CACHE = {}


def get_nc(wt):
    if wt not in _NC_CACHE:
        _NC_CACHE[wt] = build_nc(wt)
    return _NC_CACHE[wt]


def kernel(**inputs):
    wt, in_maps = prepare_inputs(**inputs)
    nc = get_nc(wt)
    res = run_bass_kernel_spmd(nc, in_maps, list(range(B)))
    out = np.stack([res.results[b]["out"] for b in range(B)], axis=0)
    return out.astype(np.float32)

